# revision 69
# baseline (speedup 1.0000x reference)
"""Trainium2 Bass kernel for GQA attention (nn_Attention_12197707121071).

Tensor-parallel across heads over 8 NeuronCores, v2:
  - Each core owns 2 query heads. KV projection is pair-split: the even core
    of each pair projects K, the odd core projects V (uniform program, the
    per-core weight input "wkv" holds wk on even cores / wv on odd cores);
    the raw projections are exchanged with a 2-rank AllGather and RoPE is
    applied to K after the exchange on both cores.
  - Attention is computed transposed (S^T = K @ Q^T per 128-key block) with
    key blocks PAIRED into [128,1024] PSUM tiles so one exp ACTIVATE covers
    two blocks; the softmax denominator comes from DVE accumulation of the
    exp tiles plus a single ones-matmul per 512-row query chunk.
  - Attention outputs are resharded head->rows with one 1MB AllToAll per
    batch (instead of 8 x 2MB AllGathers); the output projection is
    row-sharded: each core computes its 256 rows x full 2048 columns per
    batch in natural orientation (stationary = gathered activations,
    moving = full-width wo rows -> 512-free matmuls, 1 LDW per 4 matmuls).
  - Output stored bf16, host casts to fp32.
"""

import sys
import numpy as np

for _p in (
    "/root/.axon_site",
    "/root/.axon_site/_ro/trn_rl_repo",
    "/root/.axon_site/_ro/pypackages",
    "/opt/trn_rl_repo",
):
    if _p not in sys.path:
        sys.path.append(_p)

import ml_dtypes

BF16 = ml_dtypes.bfloat16

B, S, DIM = 2, 2048, 2048
N_HEADS = 16
N_KV_HEADS = 4
HEAD_DIM = 128
N_CORES = 8
PE_N = 512  # moving-operand free dim per matmul


def build_nc(b=B, s=S, debug=False):
    """Build + compile the SPMD Bass graph (identical on all 8 cores)."""
    from contextlib import ExitStack

    from concourse import bacc, mybir
    import concourse.tile as tile

    dt = mybir.dt
    f32, bf16 = dt.float32, dt.bfloat16
    rows = b * s
    KC = DIM // 128          # contraction chunks (16)
    RCB = s // PE_N          # row chunks per batch (4)
    NKB = s // 128           # key blocks per batch (16)
    NF = DIM // 128          # feature chunks for out-proj (16)
    RPC = s // N_CORES       # rows per core per batch (256)

    nc = bacc.Bacc("TRN2", target_bir_lowering=False, debug=False,
                   num_devices=N_CORES)

    d = {}
    d["xT"] = nc.dram_tensor("xT", [DIM, rows], bf16, kind="ExternalInput")
    d["wq"] = nc.dram_tensor("wq", [DIM, 256], bf16, kind="ExternalInput")
    d["wkv"] = nc.dram_tensor("wkv", [DIM, 128], bf16, kind="ExternalInput")
    d["wo"] = nc.dram_tensor("wo", [DIM, DIM], bf16, kind="ExternalInput")
    d["cosF"] = nc.dram_tensor("cosF", [128, s], bf16, kind="ExternalInput")
    d["sinPM"] = nc.dram_tensor("sinPM", [128, s], bf16, kind="ExternalInput")
    d["tri"] = nc.dram_tensor("tri", [128, 128], bf16, kind="ExternalInput")
    d["tri2"] = nc.dram_tensor("tri2", [128, 256], bf16, kind="ExternalInput")
    d["onesw"] = nc.dram_tensor("onesw", [128, 128], bf16, kind="ExternalInput")
    d["ident"] = nc.dram_tensor("ident", [128, 128], bf16, kind="ExternalInput")
    d["out"] = nc.dram_tensor("out", [b, RPC, DIM], bf16, kind="ExternalOutput")
    if debug:
        d["qdbg"] = nc.dram_tensor("qdbg", [128, 2, s], bf16, kind="ExternalOutput")
        d["kdbg"] = nc.dram_tensor("kdbg", [128, s], bf16, kind="ExternalOutput")
        d["vndbg"] = nc.dram_tensor("vndbg", [128, NKB, 128], bf16, kind="ExternalOutput")
        d["otdbg"] = nc.dram_tensor("otdbg", [2, 128, s], bf16, kind="ExternalOutput")
        d["a2aidbg"] = nc.dram_tensor("a2aidbg", [N_CORES, 2, 128, RPC], bf16,
                                      kind="ExternalOutput")
        d["a2aodbg"] = nc.dram_tensor("a2aodbg", [N_CORES, 2, 128, RPC], bf16,
                                      kind="ExternalOutput")

    # pair-AG bounce/gather for the kv exchange (per batch)
    kvb = [nc.dram_tensor(f"kvb{bi}", [128, s], bf16) for bi in range(b)]
    kvg = [nc.dram_tensor(f"kvg{bi}", [2, 128, s], bf16) for bi in range(b)]
    pair = [[0, 1], [2, 3], [4, 5], [6, 7]]
    # AllToAll buffers (per batch): block j = my 256 dims x rows j*256..
    a2a_in = [nc.dram_tensor(f"a2ain{bi}", [N_CORES, 2, 128, RPC], bf16)
              for bi in range(b)]
    a2a_out = [nc.dram_tensor(f"a2aout{bi}", [N_CORES, 2, 128, RPC], bf16)
               for bi in range(b)]

    Exp = mybir.ActivationFunctionType.Exp

    with tile.TileContext(nc) as tc, ExitStack() as ctx:
        cpool = ctx.enter_context(tc.tile_pool(name="consts", bufs=1))
        ppool = ctx.enter_context(tc.tile_pool(name="ps", bufs=1, space="PSUM"))

        # ---- persistent constants
        wq_sb = cpool.tile([128, KC, 256], bf16, tag="wq")
        wkv_sb = cpool.tile([128, KC, 128], bf16, tag="wkv")
        cos_sb = cpool.tile([128, s], bf16, tag="cos")
        sin_sb = cpool.tile([128, s], bf16, tag="sin")
        tri_sb = cpool.tile([128, 128], bf16, tag="tri")
        tri2_sb = cpool.tile([128, 256], bf16, tag="tri2")
        ones_sb = cpool.tile([128, 128], bf16, tag="ones")
        id_sb = cpool.tile([128, 128], bf16, tag="id")

        def load_wkv():
            nc.sync.dma_start(
                out=wkv_sb[:],
                in_=d["wkv"].ap().rearrange("(kc p) f -> p kc f", p=128))

        def load_wq(q0):
            nc.sync.dma_start(
                out=wq_sb[:, q0:q0 + 4, :],
                in_=d["wq"].ap().rearrange("(kc p) f -> p kc f", p=128)[:, q0:q0 + 4, :])

        def load_tables():
            nc.sync.dma_start(out=cos_sb[:], in_=d["cosF"][:, :])
            nc.sync.dma_start(out=sin_sb[:], in_=d["sinPM"][:, :])
            nc.sync.dma_start(out=tri_sb[:], in_=d["tri"][:, :])
            nc.sync.dma_start(out=tri2_sb[:], in_=d["tri2"][:, :])
            nc.sync.dma_start(out=ones_sb[:], in_=d["onesw"][:, :])
            nc.sync.dma_start(out=id_sb[:], in_=d["ident"][:, :])

        with ExitStack() as p12:
            # ---- working pools (scope closes before out-proj pools open)
            xpool = p12.enter_context(tc.tile_pool(name="xc", bufs=1))
            apool = p12.enter_context(tc.tile_pool(name="acts", bufs=1))
            tpool = p12.enter_context(tc.tile_pool(name="tmps", bufs=1))
            epool = p12.enter_context(tc.tile_pool(name="exps", bufs=1))

            # persistent-ish activations (2 batches live)
            q_sb = [apool.tile([128, 2, s], bf16, tag="q", bufs=2,
                               name=f"q{bi}") for bi in range(b)]
            kT_sb = [apool.tile([128, s], bf16, tag="k", bufs=2,
                                name=f"k{bi}") for bi in range(b)]
            vn_sb = [apool.tile([128, NKB, 128], bf16, tag="vn", bufs=2,
                                name=f"vn{bi}") for bi in range(b)]
            ot_sb = {}  # (bi, h) -> ship tile, claimed lazily

            xc = {}

            def load_x(bi, k0=0, k1=KC):
                for kcg in range(k0, k1):
                    t = xpool.tile([128, s], bf16, tag="xc", bufs=24,
                                   name=f"xc{bi}_{kcg}")
                    nc.sync.dma_start(
                        out=t[:],
                        in_=d["xT"][kcg * 128:(kcg + 1) * 128, bi * s:(bi + 1) * s])
                    xc[(bi, kcg)] = t

            def rope_evac(psum, dst, scol):
                """dst = rope(psum) in bf16 (sign folded into sinPM)."""
                c1 = tpool.tile([128, PE_N], bf16, tag="c1", bufs=3, name="c1")
                nc.vector.tensor_copy(c1[:], psum)
                sw = tpool.tile([128, PE_N], bf16, tag="sw", bufs=3, name="sw")
                nc.vector.tensor_copy(sw[0:64, :], c1[64:128, :])
                nc.vector.tensor_copy(sw[64:128, :], c1[0:64, :])
                m1 = tpool.tile([128, PE_N], bf16, tag="m1", bufs=3, name="m1")
                nc.vector.tensor_mul(m1[:], c1[:], cos_sb[:, scol:scol + PE_N])
                nc.vector.tensor_mul(sw[:], sw[:], sin_sb[:, scol:scol + PE_N])
                nc.vector.tensor_add(dst, m1[:], sw[:])

            kvouts = {}

            def kv_group(bi, rc0):
                if bi not in kvouts:
                    kvouts[bi] = apool.tile([128, s], bf16, tag="kvout", bufs=2,
                                            name=f"kvout{bi}")
                kvout = kvouts[bi]
                st = ppool.tile([128, 2 * PE_N], f32, tag="st", bufs=2,
                                name=f"kvp{bi}_{rc0}")
                for kcg in range(KC):
                    for j in range(2):
                        rc = rc0 + j
                        nc.tensor.matmul(
                            st[:, j * PE_N:(j + 1) * PE_N], wkv_sb[:, kcg, :],
                            xc[(bi, kcg)][:, rc * PE_N:(rc + 1) * PE_N],
                            start=(kcg == 0), stop=(kcg == KC - 1))
                for j in range(2):
                    rc = rc0 + j
                    nc.vector.tensor_copy(
                        kvout[:, rc * PE_N:(rc + 1) * PE_N],
                        st[:, j * PE_N:(j + 1) * PE_N])

            def kv_ship(bi):
                nc.sync.dma_start(out=kvb[bi][:, :], in_=kvouts[bi][:])
                nc.gpsimd.collective_compute(
                    "AllGather", mybir.AluOpType.bypass,
                    replica_groups=pair,
                    ins=[kvb[bi].ap().opt()],
                    outs=[kvg[bi].ap().opt()])

            def q_group(bi, mb, rc0):
                st = ppool.tile([128, 2 * PE_N], f32, tag="st", bufs=2,
                                name=f"qp{bi}_{mb}_{rc0}")
                for kcg in range(KC):
                    w_ap = wq_sb[:, kcg, mb * 128:(mb + 1) * 128]
                    for j in range(2):
                        rc = rc0 + j
                        nc.tensor.matmul(
                            st[:, j * PE_N:(j + 1) * PE_N], w_ap,
                            xc[(bi, kcg)][:, rc * PE_N:(rc + 1) * PE_N],
                            start=(kcg == 0), stop=(kcg == KC - 1))
                for j in range(2):
                    rc = rc0 + j
                    rope_evac(st[:, j * PE_N:(j + 1) * PE_N],
                              q_sb[bi][:, mb, rc * PE_N:(rc + 1) * PE_N],
                              rc * PE_N)

            def kv_part(bi):
                for rc0 in range(0, RCB, 2):
                    kv_group(bi, rc0)
                kv_ship(bi)

            def q_part(bi):
                for mb in range(2):
                    for rc0 in range(0, RCB, 2):
                        q_group(bi, mb, rc0)

            def finish_kv(bi):
                """Post-AG: load kT/vT (gpsimd DMA queue, off the busy sync
                queue) and transpose v to natural layout.  k-RoPE happens
                lazily per 512-chunk in attention prologues (rope_k_chunk)."""
                vt = apool.tile([128, s], bf16, tag="vt", bufs=2,
                                name=f"vt{bi}")
                for c0 in range(0, s, PE_N):
                    nc.gpsimd.dma_start(out=kT_sb[bi][:, c0:c0 + PE_N],
                                        in_=kvg[bi][0, :, c0:c0 + PE_N])
                    nc.gpsimd.dma_start(out=vt[:, c0:c0 + PE_N],
                                        in_=kvg[bi][1, :, c0:c0 + PE_N])
                    for kb in range(c0 // 128, c0 // 128 + 4):
                        tt = ppool.tile([128, 128], bf16, tag="od", bufs=2,
                                        name=f"tt{bi}_{kb}")
                        nc.tensor.transpose(tt[:], vt[:, kb * 128:(kb + 1) * 128],
                                            id_sb[:])
                        nc.scalar.copy(vn_sb[bi][:, kb, :], tt[:])

            def rope_k_chunk(bi, qc):
                c0 = qc * PE_N
                ksrc = kT_sb[bi][:, c0:c0 + PE_N]
                sw = tpool.tile([128, PE_N], bf16, tag="ksw", bufs=2,
                                name="ksw")
                nc.vector.tensor_copy(sw[0:64, :], ksrc[64:128, :])
                nc.vector.tensor_copy(sw[64:128, :], ksrc[0:64, :])
                m1 = tpool.tile([128, PE_N], bf16, tag="km1", bufs=2,
                                name="km1")
                nc.vector.tensor_mul(m1[:], ksrc, cos_sb[:, c0:c0 + PE_N])
                nc.vector.tensor_mul(sw[:], sw[:], sin_sb[:, c0:c0 + PE_N])
                nc.vector.tensor_add(ksrc, m1[:], sw[:])

            def attn(bi, h):
                """Transposed causal attention for one (batch, head).

                Full key-block pairs share a [128,1024] PSUM tile and one exp
                ACTIVATE; the 4 diagonal blocks are single 512-wide units
                masked with tri.  Softmax denominator: DVE adds into acc,
                folded by one ones-matmul per 512-row query chunk."""
                oth = apool.tile([128, s], bf16, tag="ot", bufs=2,
                                 name=f"ot{bi}_{h}")
                ot_sb[(bi, h)] = oth
                for qc in range(RCB):
                    if h == 0:
                        rope_k_chunk(bi, qc)
                    nkb = (qc + 1) * (PE_N // 128)
                    od = ppool.tile([128, 2 * PE_N], f32, tag="od", bufs=2,
                                    name=f"od{bi}_{h}_{qc}")
                    qs = q_sb[bi][:, h, qc * PE_N:(qc + 1) * PE_N]
                    acc = tpool.tile([128, PE_N], bf16, tag="accD", bufs=2,
                                     name="acc")
                    nc.vector.memset(acc[:], 0.0)
                    nfull = 4 * qc
                    units = [("p", kb) for kb in range(0, nfull, 2)]
                    units += [("s", kb) for kb in range(nfull, nkb)]
                    exs = {}

                    def issue_st(u):
                        kind, kb = u
                        st = ppool.tile([128, 2 * PE_N], f32, tag="st", bufs=2,
                                        name=f"st{kb}")
                        ex = epool.tile([128, 2 * PE_N], bf16, tag="ex", bufs=4,
                                        name=f"ex{kb}")
                        if kind == "p":
                            for j in range(2):
                                nc.tensor.matmul(
                                    st[:, j * PE_N:(j + 1) * PE_N],
                                    kT_sb[bi][:, (kb + j) * 128:(kb + j + 1) * 128],
                                    qs, start=True, stop=True)
                            nc.scalar.activation(ex[:, 0:2 * PE_N],
                                                 st[:, 0:2 * PE_N], Exp)
                            nc.vector.tensor_add(acc[:], acc[:], ex[:, 0:PE_N])
                            nc.vector.tensor_add(acc[:], acc[:],
                                                 ex[:, PE_N:2 * PE_N])
                        else:
                            off = (kb - nfull) * 128
                            nc.tensor.matmul(
                                st[:, off:PE_N],
                                kT_sb[bi][:, kb * 128:(kb + 1) * 128],
                                qs[:, off:], start=True, stop=True)
                            nc.scalar.activation(ex[:, off:PE_N],
                                                 st[:, off:PE_N], Exp)
                            nc.vector.tensor_mul(ex[:, off:off + 128],
                                                 ex[:, off:off + 128], tri_sb[:])
                            nc.vector.tensor_add(acc[:, off:], acc[:, off:],
                                                 ex[:, off:PE_N])
                        exs[u] = (st, ex)

                    def issue_pv(u):
                        kind, kb = u
                        st, ex = exs.pop(u)
                        first = (kb == 0)
                        if kind == "p":
                            for j in range(2):
                                nc.tensor.matmul(
                                    od[:, 0:PE_N], vn_sb[bi][:, kb + j, :],
                                    ex[:, j * PE_N:(j + 1) * PE_N],
                                    start=(first and j == 0),
                                    stop=(kb + j == nkb - 1))
                        else:
                            off = (kb - nfull) * 128
                            nc.tensor.matmul(
                                od[:, off:PE_N], vn_sb[bi][:, kb, :],
                                ex[:, off:PE_N],
                                start=first, stop=(kb == nkb - 1))

                    DEPTH = 2
                    for i, u in enumerate(units):
                        issue_st(u)
                        if i >= DEPTH:
                            issue_pv(units[i - DEPTH])
                    for u in units[max(0, len(units) - DEPTH):]:
                        issue_pv(u)

                    nc.tensor.matmul(od[:, PE_N:2 * PE_N], ones_sb[:], acc[:],
                                     start=True, stop=True)
                    rc_t = tpool.tile([128, PE_N], f32, tag="rc", bufs=2,
                                      name="rc_t")
                    nc.vector.reciprocal_approx_fast(out=rc_t[:],
                                                     in_=od[:, PE_N:2 * PE_N])
                    nc.vector.tensor_mul(oth[:, qc * PE_N:(qc + 1) * PE_N],
                                         od[:, 0:PE_N], rc_t[:])

            def ship_a2a(bi):
                for h in range(2):
                    nc.sync.dma_start(
                        out=a2a_in[bi].ap()[:, h, :, :].rearrange(
                            "j p r -> p j r"),
                        in_=ot_sb[(bi, h)].rearrange(
                            "p (j r) -> p j r", j=N_CORES))
                nc.gpsimd.collective_compute(
                    "AllToAll", mybir.AluOpType.bypass,
                    replica_groups=[list(range(N_CORES))],
                    ins=[a2a_in[bi].ap().opt()],
                    outs=[a2a_out[bi].ap().opt()])

            # ---------------- schedule ----------------
            # interleave weight/x DMAs so the first kv matmul starts early
            load_wq(0)
            load_x(0, 0, 4)
            load_wq(4)
            load_x(0, 4, 8)
            load_wq(8)
            load_wq(12)
            load_x(0, 8, 12)
            load_wkv()
            load_tables()
            load_x(0, 12, 16)
            q_part(0)
            kv_part(0)
            load_x(1)
            q_part(1)
            kv_part(1)
            finish_kv(0)
            attn(0, 0)
            attn(0, 1)
            ship_a2a(0)
            finish_kv(1)
            attn(1, 0)
            attn(1, 1)
            ship_a2a(1)
            if debug:
                nc.sync.dma_start(out=d["qdbg"].ap(), in_=q_sb[0][:, :, :])
                nc.sync.dma_start(out=d["kdbg"].ap(), in_=kT_sb[0][:, :])
                nc.sync.dma_start(out=d["vndbg"].ap(), in_=vn_sb[0][:, :, :])
                for h in range(2):
                    nc.sync.dma_start(out=d["otdbg"].ap()[h, :, :],
                                      in_=ot_sb[(0, h)][:, :])
                nc.sync.dma_start(out=d["a2aidbg"].ap(), in_=a2a_in[0].ap())
                nc.sync.dma_start(out=d["a2aodbg"].ap(), in_=a2a_out[0].ap())

        # ---------------- out-projection (row-sharded, natural) ----------
        with ExitStack() as p3:
            gpool = p3.enter_context(tc.tile_pool(name="gt", bufs=1))
            wpool = p3.enter_context(tc.tile_pool(name="wo", bufs=1))
            ospool = p3.enter_context(tc.tile_pool(name="os", bufs=4))

            wo_sb = [wpool.tile([128, DIM], bf16, tag="wof", bufs=NF,
                                name=f"wo{f}") for f in range(NF)]
            for f in range(NF):
                nc.gpsimd.dma_start(out=wo_sb[f][:],
                                    in_=d["wo"][f * 128:(f + 1) * 128, :])

            for bi in range(b):
                gts = []
                for f in range(NF):
                    gt = gpool.tile([128, RPC], bf16, tag="g", bufs=2 * NF,
                                    name=f"g{bi}_{f}")
                    nc.gpsimd.dma_start(
                        out=gt[:],
                        in_=a2a_out[bi].ap().rearrange(
                            "j t p r -> (j t) p r")[f, :, :])
                    gts.append(gt)
                for blk in range(RPC // 128):
                    pss = [ppool.tile([128, 2 * PE_N], f32, tag="st", bufs=2,
                                      name=f"po{bi}_{blk}_{i}") for i in range(2)]
                    for f in range(NF):
                        for cg in range(4):
                            nc.tensor.matmul(
                                pss[cg // 2][:, (cg % 2) * PE_N:(cg % 2 + 1) * PE_N],
                                gts[f][:, blk * 128:(blk + 1) * 128],
                                wo_sb[f][:, cg * PE_N:(cg + 1) * PE_N],
                                start=(f == 0), stop=(f == NF - 1))
                    for cg in range(4):
                        osb = ospool.tile([128, PE_N], bf16, tag="os", name="osb")
                        nc.scalar.copy(osb[:], pss[cg // 2][:, (cg % 2) * PE_N:
                                                            (cg % 2 + 1) * PE_N])
                        nc.sync.dma_start(
                            out=d["out"].ap()[bi, blk * 128:(blk + 1) * 128,
                                              cg * PE_N:(cg + 1) * PE_N],
                            in_=osb[:])

    nc.compile()
    return nc


# --------------------------------------------------------------------------
# host-side input prep / output assembly
# --------------------------------------------------------------------------

def prep_in_maps(x, freqs_cos, freqs_sin, wq, wk, wv, wo, b=B, s=S):
    """Shard + preprocess full fp32 inputs into 8 per-core input dicts."""
    n_heads = N_HEADS
    n_kv = N_KV_HEADS
    rows = b * s
    x = np.asarray(x, np.float32)
    xT = np.ascontiguousarray(x.reshape(rows, DIM).T).astype(BF16)

    sc = float(HEAD_DIM) ** -0.25
    perm64 = np.concatenate([np.arange(0, 128, 2), np.arange(1, 128, 2)])
    qcols = np.concatenate([h * 128 + perm64 for h in range(n_heads)])
    kcols = np.concatenate([g * 128 + perm64 for g in range(n_kv)])
    wq_p = (np.asarray(wq, np.float32) * sc)[:, qcols].astype(BF16)
    wk_p = (np.asarray(wk, np.float32) * sc)[:, kcols].astype(BF16)
    wv_p = np.asarray(wv, np.float32).astype(BF16)
    wo_p = np.asarray(wo, np.float32).astype(BF16)

    cosT = np.asarray(freqs_cos, np.float32).T  # (64, s)
    sinT = np.asarray(freqs_sin, np.float32).T
    cosF = np.ascontiguousarray(np.concatenate([cosT, cosT], axis=0)).astype(BF16)
    sinPM = np.ascontiguousarray(np.concatenate([-sinT, sinT], axis=0)).astype(BF16)

    qi = np.arange(128)[None, :]
    ki = np.arange(128)[:, None]
    tri = (qi >= ki).astype(BF16)
    tri2 = np.concatenate([np.zeros((128, 128), BF16), tri], axis=1)
    onesw = np.ones((128, 128), BF16)
    ident = np.eye(128, dtype=BF16)

    in_maps = []
    for c in range(N_CORES):
        g = c // 2
        wkv = wk_p[:, g * 128:(g + 1) * 128] if c % 2 == 0 \
            else wv_p[:, g * 128:(g + 1) * 128]
        in_maps.append({
            "xT": xT,
            "wq": np.ascontiguousarray(wq_p[:, c * 256:(c + 1) * 256]),
            "wkv": np.ascontiguousarray(wkv),
            "wo": wo_p,
            "cosF": cosF,
            "sinPM": sinPM,
            "tri": tri,
            "tri2": tri2,
            "onesw": onesw,
            "ident": ident,
        })
    return in_maps


def assemble_output(results, b=B, s=S):
    rpc = s // N_CORES
    out = np.empty((b, s, DIM), np.float32)
    for c in range(N_CORES):
        out[:, c * rpc:(c + 1) * rpc, :] = results[c]["out"].astype(np.float32)
    return out


_NC_CACHE = {}


def _get_nc(b=B, s=S, debug=False):
    key = (b, s, debug)
    if key not in _NC_CACHE:
        _NC_CACHE[key] = build_nc(b, s, debug)
    return _NC_CACHE[key]


def run(inputs, trace=False, b=B, s=S, debug=False):
    """Run the kernel; returns (output, BassKernelResults)."""
    from concourse import bass_utils
    nc = _get_nc(b, s, debug)
    in_maps = prep_in_maps(**inputs, b=b, s=s)
    res = bass_utils.run_bass_kernel_spmd(
        nc, in_maps, core_ids=list(range(N_CORES)), trace=trace)
    return assemble_output(res.results, b, s), res


def kernel(**inputs):
    out, _ = run(inputs)
    return out


# revision 70
# speedup vs baseline: 1.0887x; 1.0887x over previous
"""Trainium2 Bass kernel for GQA attention (nn_Attention_12197707121071).

Tensor-parallel across heads over 8 NeuronCores, v2:
  - Each core owns 2 query heads. KV projection is pair-split: the even core
    of each pair projects K, the odd core projects V (uniform program, the
    per-core weight input "wkv" holds wk on even cores / wv on odd cores);
    the raw projections are exchanged with a 2-rank AllGather and RoPE is
    applied to K after the exchange on both cores.
  - Attention is computed transposed (S^T = K @ Q^T per 128-key block) with
    key blocks PAIRED into [128,1024] PSUM tiles so one exp ACTIVATE covers
    two blocks; the softmax denominator comes from DVE accumulation of the
    exp tiles plus a single ones-matmul per 512-row query chunk.
  - Attention outputs are resharded head->rows with one 1MB AllToAll per
    batch (instead of 8 x 2MB AllGathers); the output projection is
    row-sharded: each core computes its 256 rows x full 2048 columns per
    batch in natural orientation (stationary = gathered activations,
    moving = full-width wo rows -> 512-free matmuls, 1 LDW per 4 matmuls).
  - Output stored bf16, host casts to fp32.
"""

import sys
import numpy as np

for _p in (
    "/root/.axon_site",
    "/root/.axon_site/_ro/trn_rl_repo",
    "/root/.axon_site/_ro/pypackages",
    "/opt/trn_rl_repo",
):
    if _p not in sys.path:
        sys.path.append(_p)

import ml_dtypes

BF16 = ml_dtypes.bfloat16

B, S, DIM = 2, 2048, 2048
N_HEADS = 16
N_KV_HEADS = 4
HEAD_DIM = 128
N_CORES = 8
PE_N = 512  # moving-operand free dim per matmul


def build_nc(b=B, s=S, debug=False):
    """Build + compile the SPMD Bass graph (identical on all 8 cores)."""
    from contextlib import ExitStack

    from concourse import bacc, mybir
    import concourse.tile as tile

    dt = mybir.dt
    f32, bf16 = dt.float32, dt.bfloat16
    rows = b * s
    KC = DIM // 128          # contraction chunks (16)
    RCB = s // PE_N          # row chunks per batch (4)
    NKB = s // 128           # key blocks per batch (16)
    NF = DIM // 128          # feature chunks for out-proj (16)
    RPC = s // N_CORES       # rows per core per batch (256)

    nc = bacc.Bacc("TRN2", target_bir_lowering=False, debug=False,
                   num_devices=N_CORES)

    d = {}
    d["xT"] = nc.dram_tensor("xT", [DIM, rows], bf16, kind="ExternalInput")
    d["wq"] = nc.dram_tensor("wq", [DIM, 256], bf16, kind="ExternalInput")
    d["wkv"] = nc.dram_tensor("wkv", [DIM, 128], bf16, kind="ExternalInput")
    d["wo"] = nc.dram_tensor("wo", [DIM, DIM], bf16, kind="ExternalInput")
    d["cosF"] = nc.dram_tensor("cosF", [128, s], bf16, kind="ExternalInput")
    d["sinPM"] = nc.dram_tensor("sinPM", [128, s], bf16, kind="ExternalInput")
    d["tri"] = nc.dram_tensor("tri", [128, 128], bf16, kind="ExternalInput")
    d["tri2"] = nc.dram_tensor("tri2", [128, 256], bf16, kind="ExternalInput")
    d["onesw"] = nc.dram_tensor("onesw", [128, 128], bf16, kind="ExternalInput")
    d["ident"] = nc.dram_tensor("ident", [128, 128], bf16, kind="ExternalInput")
    d["out"] = nc.dram_tensor("out", [b, RPC, DIM], bf16, kind="ExternalOutput")
    if debug:
        d["qdbg"] = nc.dram_tensor("qdbg", [128, 2, s], bf16, kind="ExternalOutput")
        d["kdbg"] = nc.dram_tensor("kdbg", [128, s], bf16, kind="ExternalOutput")
        d["vndbg"] = nc.dram_tensor("vndbg", [128, NKB, 128], bf16, kind="ExternalOutput")
        d["otdbg"] = nc.dram_tensor("otdbg", [2, 128, s], bf16, kind="ExternalOutput")
        d["a2aidbg"] = nc.dram_tensor("a2aidbg", [N_CORES, 2, 128, RPC], bf16,
                                      kind="ExternalOutput")
        d["a2aodbg"] = nc.dram_tensor("a2aodbg", [N_CORES, 2, 128, RPC], bf16,
                                      kind="ExternalOutput")

    # pair-AG bounce/gather for the kv exchange (per batch)
    kvb = [nc.dram_tensor(f"kvb{bi}", [128, s], bf16) for bi in range(b)]
    kvg = [nc.dram_tensor(f"kvg{bi}", [2, 128, s], bf16) for bi in range(b)]
    pair = [[0, 1], [2, 3], [4, 5], [6, 7]]
    # AllToAll buffers (per batch): block j = my 256 dims x rows j*256..
    a2a_in = [nc.dram_tensor(f"a2ain{bi}", [N_CORES, 2, 128, RPC], bf16)
              for bi in range(b)]
    a2a_out = [nc.dram_tensor(f"a2aout{bi}", [N_CORES, 2, 128, RPC], bf16)
               for bi in range(b)]

    Exp = mybir.ActivationFunctionType.Exp

    with tile.TileContext(nc) as tc, ExitStack() as ctx:
        cpool = ctx.enter_context(tc.tile_pool(name="consts", bufs=1))
        ppool = ctx.enter_context(tc.tile_pool(name="ps", bufs=1, space="PSUM"))

        # ---- persistent constants
        wq_sb = cpool.tile([128, KC, 256], bf16, tag="wq")
        wkv_sb = cpool.tile([128, KC, 128], bf16, tag="wkv")
        cos_sb = cpool.tile([128, s], bf16, tag="cos")
        sin_sb = cpool.tile([128, s], bf16, tag="sin")
        tri_sb = cpool.tile([128, 128], bf16, tag="tri")
        tri2_sb = cpool.tile([128, 256], bf16, tag="tri2")
        ones_sb = cpool.tile([128, 128], bf16, tag="ones")
        id_sb = cpool.tile([128, 128], bf16, tag="id")

        def load_wkv():
            nc.sync.dma_start(
                out=wkv_sb[:],
                in_=d["wkv"].ap().rearrange("(kc p) f -> p kc f", p=128))

        def load_wq(q0):
            nc.sync.dma_start(
                out=wq_sb[:, q0:q0 + 4, :],
                in_=d["wq"].ap().rearrange("(kc p) f -> p kc f", p=128)[:, q0:q0 + 4, :])

        def load_tables():
            nc.sync.dma_start(out=cos_sb[:], in_=d["cosF"][:, :])
            nc.sync.dma_start(out=sin_sb[:], in_=d["sinPM"][:, :])
            nc.sync.dma_start(out=tri_sb[:], in_=d["tri"][:, :])
            nc.sync.dma_start(out=tri2_sb[:], in_=d["tri2"][:, :])
            nc.sync.dma_start(out=ones_sb[:], in_=d["onesw"][:, :])
            nc.sync.dma_start(out=id_sb[:], in_=d["ident"][:, :])

        with ExitStack() as p12:
            # ---- working pools (scope closes before out-proj pools open)
            xpool = p12.enter_context(tc.tile_pool(name="xc", bufs=1))
            apool = p12.enter_context(tc.tile_pool(name="acts", bufs=1))
            tpool = p12.enter_context(tc.tile_pool(name="tmps", bufs=1))
            epool = p12.enter_context(tc.tile_pool(name="exps", bufs=1))

            # persistent-ish activations (2 batches live)
            q_sb = [apool.tile([128, 2, s], bf16, tag="q", bufs=2,
                               name=f"q{bi}") for bi in range(b)]
            kT_sb = [apool.tile([128, s], bf16, tag="k", bufs=2,
                                name=f"k{bi}") for bi in range(b)]
            vn_sb = [apool.tile([128, NKB, 128], bf16, tag="vn", bufs=2,
                                name=f"vn{bi}") for bi in range(b)]
            ot_sb = {}  # (bi, h) -> ship tile, claimed lazily

            xc = {}

            def load_x(bi, k0=0, k1=KC):
                for kcg in range(k0, k1):
                    t = xpool.tile([128, s], bf16, tag="xc", bufs=24,
                                   name=f"xc{bi}_{kcg}")
                    nc.sync.dma_start(
                        out=t[:],
                        in_=d["xT"][kcg * 128:(kcg + 1) * 128, bi * s:(bi + 1) * s])
                    xc[(bi, kcg)] = t

            def rope_evac(psum, dst, scol):
                """dst = rope(psum) in bf16 (sign folded into sinPM)."""
                c1 = tpool.tile([128, PE_N], bf16, tag="c1", bufs=3, name="c1")
                nc.vector.tensor_copy(c1[:], psum)
                sw = tpool.tile([128, PE_N], bf16, tag="sw", bufs=3, name="sw")
                nc.vector.tensor_copy(sw[0:64, :], c1[64:128, :])
                nc.vector.tensor_copy(sw[64:128, :], c1[0:64, :])
                m1 = tpool.tile([128, PE_N], bf16, tag="m1", bufs=3, name="m1")
                nc.vector.tensor_mul(m1[:], c1[:], cos_sb[:, scol:scol + PE_N])
                nc.vector.tensor_mul(sw[:], sw[:], sin_sb[:, scol:scol + PE_N])
                nc.vector.tensor_add(dst, m1[:], sw[:])

            kvouts = {}

            def kv_group(bi, rc0):
                if bi not in kvouts:
                    kvouts[bi] = apool.tile([128, s], bf16, tag="kvout", bufs=2,
                                            name=f"kvout{bi}")
                kvout = kvouts[bi]
                st = ppool.tile([128, 2 * PE_N], f32, tag="st", bufs=2,
                                name=f"kvp{bi}_{rc0}")
                for kcg in range(KC):
                    for j in range(2):
                        rc = rc0 + j
                        nc.tensor.matmul(
                            st[:, j * PE_N:(j + 1) * PE_N], wkv_sb[:, kcg, :],
                            xc[(bi, kcg)][:, rc * PE_N:(rc + 1) * PE_N],
                            start=(kcg == 0), stop=(kcg == KC - 1))
                for j in range(2):
                    rc = rc0 + j
                    nc.vector.tensor_copy(
                        kvout[:, rc * PE_N:(rc + 1) * PE_N],
                        st[:, j * PE_N:(j + 1) * PE_N])

            def kv_ship(bi):
                nc.sync.dma_start(out=kvb[bi][:, :], in_=kvouts[bi][:])
                nc.gpsimd.collective_compute(
                    "AllGather", mybir.AluOpType.bypass,
                    replica_groups=pair,
                    ins=[kvb[bi].ap().opt()],
                    outs=[kvg[bi].ap().opt()])

            def q_group(bi, mb, rc0):
                st = ppool.tile([128, 2 * PE_N], f32, tag="st", bufs=2,
                                name=f"qp{bi}_{mb}_{rc0}")
                for kcg in range(KC):
                    w_ap = wq_sb[:, kcg, mb * 128:(mb + 1) * 128]
                    for j in range(2):
                        rc = rc0 + j
                        nc.tensor.matmul(
                            st[:, j * PE_N:(j + 1) * PE_N], w_ap,
                            xc[(bi, kcg)][:, rc * PE_N:(rc + 1) * PE_N],
                            start=(kcg == 0), stop=(kcg == KC - 1))
                for j in range(2):
                    rc = rc0 + j
                    rope_evac(st[:, j * PE_N:(j + 1) * PE_N],
                              q_sb[bi][:, mb, rc * PE_N:(rc + 1) * PE_N],
                              rc * PE_N)

            def kv_part(bi):
                for rc0 in range(0, RCB, 2):
                    kv_group(bi, rc0)
                kv_ship(bi)

            def q_part(bi):
                for mb in range(2):
                    for rc0 in range(0, RCB, 2):
                        q_group(bi, mb, rc0)

            def finish_kv(bi):
                """Post-AG: load kT/vT (gpsimd DMA queue, off the busy sync
                queue) and transpose v to natural layout.  k-RoPE happens
                lazily per 512-chunk in attention prologues (rope_k_chunk)."""
                vt = apool.tile([128, s], bf16, tag="vt", bufs=2,
                                name=f"vt{bi}")
                for c0 in range(0, s, PE_N):
                    nc.gpsimd.dma_start(out=kT_sb[bi][:, c0:c0 + PE_N],
                                        in_=kvg[bi][0, :, c0:c0 + PE_N])
                    nc.gpsimd.dma_start(out=vt[:, c0:c0 + PE_N],
                                        in_=kvg[bi][1, :, c0:c0 + PE_N])
                    for kb in range(c0 // 128, c0 // 128 + 4):
                        tt = ppool.tile([128, 128], bf16, tag="od", bufs=2,
                                        name=f"tt{bi}_{kb}")
                        nc.tensor.transpose(tt[:], vt[:, kb * 128:(kb + 1) * 128],
                                            id_sb[:])
                        nc.scalar.copy(vn_sb[bi][:, kb, :], tt[:])

            def rope_k_chunk(bi, qc):
                c0 = qc * PE_N
                ksrc = kT_sb[bi][:, c0:c0 + PE_N]
                sw = tpool.tile([128, PE_N], bf16, tag="ksw", bufs=2,
                                name="ksw")
                nc.vector.tensor_copy(sw[0:64, :], ksrc[64:128, :])
                nc.vector.tensor_copy(sw[64:128, :], ksrc[0:64, :])
                m1 = tpool.tile([128, PE_N], bf16, tag="km1", bufs=2,
                                name="km1")
                nc.vector.tensor_mul(m1[:], ksrc, cos_sb[:, c0:c0 + PE_N])
                nc.vector.tensor_mul(sw[:], sw[:], sin_sb[:, c0:c0 + PE_N])
                nc.vector.tensor_add(ksrc, m1[:], sw[:])

            def attn(bi, h):
                """Transposed causal attention for one (batch, head).

                Full key-block pairs share a [128,1024] PSUM tile and one exp
                ACTIVATE; the 4 diagonal blocks are single 512-wide units
                masked with tri.  Softmax denominator: DVE adds into acc,
                folded by one ones-matmul per 512-row query chunk."""
                oth = apool.tile([128, s], bf16, tag="ot", bufs=2,
                                 name=f"ot{bi}_{h}")
                ot_sb[(bi, h)] = oth
                for qc in range(RCB):
                    if h == 0:
                        rope_k_chunk(bi, qc)
                    nkb = (qc + 1) * (PE_N // 128)
                    od = ppool.tile([128, 2 * PE_N], f32, tag="od", bufs=2,
                                    name=f"od{bi}_{h}_{qc}")
                    qs = q_sb[bi][:, h, qc * PE_N:(qc + 1) * PE_N]
                    acc = tpool.tile([128, PE_N], bf16, tag="accD", bufs=2,
                                     name="acc")
                    nc.vector.memset(acc[:], 0.0)
                    nfull = 4 * qc
                    units = [("p", kb) for kb in range(0, nfull, 2)]
                    units += [("s", kb) for kb in range(nfull, nkb)]
                    exs = {}

                    def issue_st(u):
                        kind, kb = u
                        st = ppool.tile([128, 2 * PE_N], f32, tag="st", bufs=2,
                                        name=f"st{kb}")
                        ex = epool.tile([128, 2 * PE_N], bf16, tag="ex", bufs=4,
                                        name=f"ex{kb}")
                        if kind == "p":
                            for j in range(2):
                                nc.tensor.matmul(
                                    st[:, j * PE_N:(j + 1) * PE_N],
                                    kT_sb[bi][:, (kb + j) * 128:(kb + j + 1) * 128],
                                    qs, start=True, stop=True)
                            nc.scalar.activation(ex[:, 0:2 * PE_N],
                                                 st[:, 0:2 * PE_N], Exp)
                            nc.vector.tensor_add(acc[:], acc[:], ex[:, 0:PE_N])
                            nc.vector.tensor_add(acc[:], acc[:],
                                                 ex[:, PE_N:2 * PE_N])
                        else:
                            off = (kb - nfull) * 128
                            nc.tensor.matmul(
                                st[:, off:PE_N],
                                kT_sb[bi][:, kb * 128:(kb + 1) * 128],
                                qs[:, off:], start=True, stop=True)
                            nc.scalar.activation(ex[:, off:PE_N],
                                                 st[:, off:PE_N], Exp)
                            nc.vector.tensor_mul(ex[:, off:off + 128],
                                                 ex[:, off:off + 128], tri_sb[:])
                            nc.vector.tensor_add(acc[:, off:], acc[:, off:],
                                                 ex[:, off:PE_N])
                        exs[u] = (st, ex)

                    def issue_pv(u):
                        kind, kb = u
                        st, ex = exs.pop(u)
                        first = (kb == 0)
                        if kind == "p":
                            for j in range(2):
                                nc.tensor.matmul(
                                    od[:, 0:PE_N], vn_sb[bi][:, kb + j, :],
                                    ex[:, j * PE_N:(j + 1) * PE_N],
                                    start=(first and j == 0),
                                    stop=(kb + j == nkb - 1))
                        else:
                            off = (kb - nfull) * 128
                            nc.tensor.matmul(
                                od[:, off:PE_N], vn_sb[bi][:, kb, :],
                                ex[:, off:PE_N],
                                start=first, stop=(kb == nkb - 1))

                    DEPTH = 2
                    for i, u in enumerate(units):
                        issue_st(u)
                        if i >= DEPTH:
                            issue_pv(units[i - DEPTH])
                    for u in units[max(0, len(units) - DEPTH):]:
                        issue_pv(u)

                    nc.tensor.matmul(od[:, PE_N:2 * PE_N], ones_sb[:], acc[:],
                                     start=True, stop=True)
                    rc_t = tpool.tile([128, PE_N], f32, tag="rc", bufs=2,
                                      name="rc_t")
                    nc.vector.reciprocal_approx_fast(out=rc_t[:],
                                                     in_=od[:, PE_N:2 * PE_N])
                    nc.vector.tensor_mul(oth[:, qc * PE_N:(qc + 1) * PE_N],
                                         od[:, 0:PE_N], rc_t[:])

            def ship_a2a(bi):
                for h in range(2):
                    nc.sync.dma_start(
                        out=a2a_in[bi].ap()[:, h, :, :].rearrange(
                            "j p r -> p j r"),
                        in_=ot_sb[(bi, h)].rearrange(
                            "p (j r) -> p j r", j=N_CORES))
                nc.gpsimd.collective_compute(
                    "AllToAll", mybir.AluOpType.bypass,
                    replica_groups=[list(range(N_CORES))],
                    ins=[a2a_in[bi].ap().opt()],
                    outs=[a2a_out[bi].ap().opt()])

            # ---------------- schedule ----------------
            # interleave weight/x DMAs so the first kv matmul starts early
            load_wkv()
            load_x(0, 0, 4)
            load_wq(0)
            load_x(0, 4, 8)
            load_wq(4)
            load_wq(8)
            load_x(0, 8, 12)
            load_wq(12)
            load_tables()
            load_x(0, 12, 16)
            q_part(0)
            kv_part(0)
            load_x(1)
            q_part(1)
            kv_part(1)
            finish_kv(0)
            attn(0, 0)
            attn(0, 1)
            ship_a2a(0)
            finish_kv(1)
            attn(1, 0)
            attn(1, 1)
            ship_a2a(1)
            if debug:
                nc.sync.dma_start(out=d["qdbg"].ap(), in_=q_sb[0][:, :, :])
                nc.sync.dma_start(out=d["kdbg"].ap(), in_=kT_sb[0][:, :])
                nc.sync.dma_start(out=d["vndbg"].ap(), in_=vn_sb[0][:, :, :])
                for h in range(2):
                    nc.sync.dma_start(out=d["otdbg"].ap()[h, :, :],
                                      in_=ot_sb[(0, h)][:, :])
                nc.sync.dma_start(out=d["a2aidbg"].ap(), in_=a2a_in[0].ap())
                nc.sync.dma_start(out=d["a2aodbg"].ap(), in_=a2a_out[0].ap())

        # ---------------- out-projection (row-sharded, natural) ----------
        with ExitStack() as p3:
            gpool = p3.enter_context(tc.tile_pool(name="gt", bufs=1))
            wpool = p3.enter_context(tc.tile_pool(name="wo", bufs=1))
            ospool = p3.enter_context(tc.tile_pool(name="os", bufs=4))

            wo_sb = [wpool.tile([128, DIM], bf16, tag="wof", bufs=NF,
                                name=f"wo{f}") for f in range(NF)]
            for f in range(NF):
                nc.gpsimd.dma_start(out=wo_sb[f][:],
                                    in_=d["wo"][f * 128:(f + 1) * 128, :])

            for bi in range(b):
                gts = []
                for f in range(NF):
                    gt = gpool.tile([128, RPC], bf16, tag="g", bufs=2 * NF,
                                    name=f"g{bi}_{f}")
                    nc.gpsimd.dma_start(
                        out=gt[:],
                        in_=a2a_out[bi].ap().rearrange(
                            "j t p r -> (j t) p r")[f, :, :])
                    gts.append(gt)
                for blk in range(RPC // 128):
                    pss = [ppool.tile([128, 2 * PE_N], f32, tag="st", bufs=2,
                                      name=f"po{bi}_{blk}_{i}") for i in range(2)]
                    for f in range(NF):
                        for cg in range(4):
                            nc.tensor.matmul(
                                pss[cg // 2][:, (cg % 2) * PE_N:(cg % 2 + 1) * PE_N],
                                gts[f][:, blk * 128:(blk + 1) * 128],
                                wo_sb[f][:, cg * PE_N:(cg + 1) * PE_N],
                                start=(f == 0), stop=(f == NF - 1))
                    for cg in range(4):
                        osb = ospool.tile([128, PE_N], bf16, tag="os", name="osb")
                        nc.scalar.copy(osb[:], pss[cg // 2][:, (cg % 2) * PE_N:
                                                            (cg % 2 + 1) * PE_N])
                        nc.sync.dma_start(
                            out=d["out"].ap()[bi, blk * 128:(blk + 1) * 128,
                                              cg * PE_N:(cg + 1) * PE_N],
                            in_=osb[:])

    nc.compile()
    return nc


# --------------------------------------------------------------------------
# host-side input prep / output assembly
# --------------------------------------------------------------------------

def prep_in_maps(x, freqs_cos, freqs_sin, wq, wk, wv, wo, b=B, s=S):
    """Shard + preprocess full fp32 inputs into 8 per-core input dicts."""
    n_heads = N_HEADS
    n_kv = N_KV_HEADS
    rows = b * s
    x = np.asarray(x, np.float32)
    xT = np.ascontiguousarray(x.reshape(rows, DIM).T).astype(BF16)

    sc = float(HEAD_DIM) ** -0.25
    perm64 = np.concatenate([np.arange(0, 128, 2), np.arange(1, 128, 2)])
    qcols = np.concatenate([h * 128 + perm64 for h in range(n_heads)])
    kcols = np.concatenate([g * 128 + perm64 for g in range(n_kv)])
    wq_p = (np.asarray(wq, np.float32) * sc)[:, qcols].astype(BF16)
    wk_p = (np.asarray(wk, np.float32) * sc)[:, kcols].astype(BF16)
    wv_p = np.asarray(wv, np.float32).astype(BF16)
    wo_p = np.asarray(wo, np.float32).astype(BF16)

    cosT = np.asarray(freqs_cos, np.float32).T  # (64, s)
    sinT = np.asarray(freqs_sin, np.float32).T
    cosF = np.ascontiguousarray(np.concatenate([cosT, cosT], axis=0)).astype(BF16)
    sinPM = np.ascontiguousarray(np.concatenate([-sinT, sinT], axis=0)).astype(BF16)

    qi = np.arange(128)[None, :]
    ki = np.arange(128)[:, None]
    tri = (qi >= ki).astype(BF16)
    tri2 = np.concatenate([np.zeros((128, 128), BF16), tri], axis=1)
    onesw = np.ones((128, 128), BF16)
    ident = np.eye(128, dtype=BF16)

    in_maps = []
    for c in range(N_CORES):
        g = c // 2
        wkv = wk_p[:, g * 128:(g + 1) * 128] if c % 2 == 0 \
            else wv_p[:, g * 128:(g + 1) * 128]
        in_maps.append({
            "xT": xT,
            "wq": np.ascontiguousarray(wq_p[:, c * 256:(c + 1) * 256]),
            "wkv": np.ascontiguousarray(wkv),
            "wo": wo_p,
            "cosF": cosF,
            "sinPM": sinPM,
            "tri": tri,
            "tri2": tri2,
            "onesw": onesw,
            "ident": ident,
        })
    return in_maps


def assemble_output(results, b=B, s=S):
    rpc = s // N_CORES
    out = np.empty((b, s, DIM), np.float32)
    for c in range(N_CORES):
        out[:, c * rpc:(c + 1) * rpc, :] = results[c]["out"].astype(np.float32)
    return out


_NC_CACHE = {}


def _get_nc(b=B, s=S, debug=False):
    key = (b, s, debug)
    if key not in _NC_CACHE:
        _NC_CACHE[key] = build_nc(b, s, debug)
    return _NC_CACHE[key]


def run(inputs, trace=False, b=B, s=S, debug=False):
    """Run the kernel; returns (output, BassKernelResults)."""
    from concourse import bass_utils
    nc = _get_nc(b, s, debug)
    in_maps = prep_in_maps(**inputs, b=b, s=s)
    res = bass_utils.run_bass_kernel_spmd(
        nc, in_maps, core_ids=list(range(N_CORES)), trace=trace)
    return assemble_output(res.results, b, s), res


def kernel(**inputs):
    out, _ = run(inputs)
    return out
